# revision 1
# baseline (speedup 1.0000x reference)
"""DCGRU cell on 8 Trainium2 NeuronCores.

Sharding: data-parallel over batch (B=32 -> 4 per core), adjacency + MLP
weights replicated. No collectives; host gathers per-core outputs.

Per-core layouts (all f32):
  node-major (nm): [16 tiles][128 nodes, 768] cols = b*192+f   (diffusion lhsT)
  feat-major (fm): [6 tiles][128 bf-rows, 2048 nodes]          (hop outputs, MLP rhs)
Hop matmul: out_fm[bf, i] = sum_j x_nm[j, bf] * W[i, j]
  = matmul(lhsT=x_nm[jt][:, c*128:+128], rhs=WT[jt][:, i-block]) accumulated
  over jt in PSUM, so W is streamed host-pretransposed (WT[j, i] = W[i, j]).
MLP: gate logits acc[b][o, n] += WxI[k][bf, o].T @ fm[k][bf-slice, n] with
  batch-interleaved host-packed weights WxI (rows = b*192+f), accumulated
  across hops in DRAM via accum_op=add DMAs straight from PSUM.
Chain re-entry: fm -> nm via PE transposes (hops 1,2 of each direction only).
"""

import sys
import numpy as np
import ml_dtypes

for _p in ("/opt/trn_rl_repo",):
    if _p not in sys.path:
        sys.path.insert(0, _p)

from concourse import bacc, tile, mybir  # noqa: E402
from concourse.alu_op_type import AluOpType as ALU  # noqa: E402
from concourse.bass_utils import run_bass_kernel_spmd  # noqa: E402

F32 = mybir.dt.float32
F32R = mybir.dt.float32r
BF16 = mybir.dt.bfloat16
MM_BF16 = True          # matmul datapath dtype: True -> bf16, False -> f32r
MMDT = BF16 if MM_BF16 else F32R
AF = mybir.ActivationFunctionType

C = 4          # batches per core
FI = 192       # per-batch feature width (x 64 + h 128)
BF = C * FI    # 768
DH = 128
NCORES = 8
NHOPS = 3


def build_nc(nt=16):
    """Build + compile the per-core Bass kernel. nt = node tiles (N = nt*128)."""
    N = nt * 128
    nbk = N // 512

    nc = bacc.Bacc("TRN2", target_bir_lowering=False, debug=False,
                   num_devices=NCORES)

    def din(name, shape, dt=F32):
        return nc.dram_tensor(name, shape, dt, kind="ExternalInput").ap()

    XH = din("xh_nm", [nt, 128, BF], MMDT)
    XHFM = din("xh_fm", [6, 128, N], MMDT)
    WFT = din("wfT", [nt, 128, N], MMDT)
    WBT = din("wbT", [nt, 128, N], MMDT)
    WRI = din("wrI", [7, 3, 64, 128], MMDT)
    WZI = din("wzI", [7, 3, 64, 128], MMDT)
    WNI = din("wnI", [7, 3, 64, 128], MMDT)
    XFM = din("x_fm", [C, 64, N], MMDT)
    HFM = din("h_fm", [C, 128, N])
    BR = din("br_c", [128, 1])
    BZ = din("bz_c", [128, 1])
    BN = din("bn_c", [128, 1])
    IDT = din("ident", [128, 128], MMDT)
    OUT = nc.dram_tensor("out_fm", [C, 128, N], F32, kind="ExternalOutput").ap()

    ACCR = nc.dram_tensor("acc_r", [C, 128, N], F32).ap()
    ACCZ = nc.dram_tensor("acc_z", [C, 128, N], F32).ap()
    ACCN = nc.dram_tensor("acc_n", [C, 128, N], F32).ap()
    XRH = nc.dram_tensor("xrh_nm_d", [nt, 128, BF], MMDT).ap()

    with tile.TileContext(nc) as tc:
        with (
            tc.tile_pool(name="nm", bufs=32) as nm_pool,
            tc.tile_pool(name="fm", bufs=12) as fm_pool,
            tc.tile_pool(name="gate", bufs=4) as gate_pool,
            tc.tile_pool(name="wt", bufs=6) as wt_pool,
            tc.tile_pool(name="wxi", bufs=18) as wxi_pool,
            tc.tile_pool(name="aux", bufs=12) as aux_pool,
            tc.tile_pool(name="stg", bufs=4) as stg_pool,
            tc.tile_pool(name="const", bufs=1) as const_pool,
            tc.tile_pool(name="ps", bufs=6, space="PSUM") as ps_pool,
            tc.tile_pool(name="psx", bufs=2, space="PSUM") as psx_pool,
        ):
            ident = const_pool.tile([128, 128], MMDT, tag="ident")
            nc.sync.dma_start(ident[:], IDT[:])
            brt = const_pool.tile([128, 1], F32, tag="brt")
            nc.sync.dma_start(brt[:], BR[:])
            bzt = const_pool.tile([128, 1], F32, tag="bzt")
            nc.sync.dma_start(bzt[:], BZ[:])
            bnt = const_pool.tile([128, 1], F32, tag="bnt")
            nc.sync.dma_start(bnt[:], BN[:])

            def load_nm(SRC):
                ts = []
                for jt in range(nt):
                    t = nm_pool.tile([128, BF], MMDT, name="nmt", tag="nm")
                    nc.sync.dma_start(t[:], SRC[jt])
                    ts.append(t)
                return ts

            def hop(src, WT):
                """One diffusion hop; returns fm tiles (6 x [128, N])."""
                fms = [fm_pool.tile([128, N], MMDT, name="fmt", tag="fm") for _ in range(6)]
                for ibk in range(nbk):
                    pss = [ps_pool.tile([128, 512], F32, name="pst", tag="ps")
                           for _ in range(6)]
                    for jt in range(nt):
                        wt = wt_pool.tile([128, 512], MMDT, name="wtt", tag="wt")
                        nc.sync.dma_start(
                            wt[:], WT[jt][:, 512 * ibk:512 * (ibk + 1)])
                        for c in range(6):
                            nc.tensor.matmul(
                                pss[c][:],
                                src[jt][:, 128 * c:128 * (c + 1)],
                                wt[:],
                                start=(jt == 0), stop=(jt == nt - 1))
                    for c in range(6):
                        nc.vector.tensor_copy(
                            fms[c][:, 512 * ibk:512 * (ibk + 1)], pss[c][:])
                return fms

            def aux_of(fms):
                """Base-0 copies of rows [64:128) of each fm tile (so every
                MLP contraction segment sits at partition 0 -> one PSUM
                accumulation group, no mixed tile_position)."""
                auxs = []
                for t in range(6):
                    a = aux_pool.tile([64, N], MMDT, name="auxt", tag="aux")
                    nc.gpsimd.dma_start(a[:], fms[t][64:128, :])
                    auxs.append(a)
                return auxs

            def mlp_feed(fms, auxs, kidx, gates, first):
                """gates: list of (WXI dram, ACC dram). Accumulate logits."""
                for WXI, ACCD in gates:
                    wx = []
                    for s in range(3):
                        w = wxi_pool.tile([64, 128], MMDT, name="wxit", tag="wxi")
                        nc.gpsimd.dma_start(w[:], WXI[kidx][s])
                        wx.append(w)
                    for b in range(C):
                        for nb in range(nbk):
                            nbs = slice(512 * nb, 512 * (nb + 1))
                            ps = psx_pool.tile([128, 512], F32, name="psxt", tag="psx")
                            for s in range(3):
                                t, off = divmod(b * FI + 64 * s, 128)
                                rhs = (fms[t][0:64, nbs] if off == 0
                                       else auxs[t][0:64, nbs])
                                nc.tensor.matmul(ps[:], wx[s][:], rhs,
                                                 start=(s == 0), stop=(s == 2))
                            stg = stg_pool.tile([128, 512], F32, name="stgt", tag="stg")
                            nc.vector.tensor_copy(stg[:], ps[:])
                            nc.gpsimd.dma_start(
                                ACCD[b][:, nbs], stg[:],
                                accum_op=(ALU.bypass if first else ALU.add))

            def retranspose(fms):
                """fm tiles -> fresh nm tiles via PE transposes."""
                nms = [nm_pool.tile([128, BF], MMDT, name="nmt", tag="nm")
                       for _ in range(nt)]
                for it in range(nt):
                    ps = psx_pool.tile([128, BF], MMDT, name="psxt", tag="psx")
                    for c in range(6):
                        nc.tensor.transpose(
                            ps[:, 128 * c:128 * (c + 1)],
                            fms[c][:, 128 * it:128 * (it + 1)],
                            ident[:])
                    nc.vector.tensor_copy(nms[it][:], ps[:])
                return nms

            def diffusion(x_nm_loader, x_fm_tiles, gates, xnm_first=None):
                """Full 2-direction diffusion + MLP accumulation.
                MLP feeds are deferred one hop so they never gate the next
                hop's matmul stream (fm pool holds 2 chunks)."""
                mlp_feed(x_fm_tiles, aux_of(x_fm_tiles), 0, gates,
                         first=True)
                pending = None
                cur = xnm_first if xnm_first is not None else x_nm_loader()
                for wdir, WT in ((0, WFT), (1, WBT)):
                    if wdir == 1:
                        cur = x_nm_loader()
                    for k in range(1, NHOPS + 1):
                        fm = hop(cur, WT)
                        aux = aux_of(fm)
                        cur = retranspose(fm) if k < NHOPS else None
                        if pending is not None:
                            mlp_feed(*pending)
                        pending = (fm, aux, wdir * NHOPS + k, gates, False)
                mlp_feed(*pending)

            # ---------------- diffusion 1 (r, z gates) ----------------
            fm0 = []
            for t in range(6):
                f = fm_pool.tile([128, N], MMDT, name="fmt", tag="fm")
                nc.scalar.dma_start(f[:], XHFM[t])
                fm0.append(f)
            diffusion(lambda: load_nm(XH), fm0, [(WRI, ACCR), (WZI, ACCZ)])

            # ------------- gates r, z; assemble xrh (nm + fm) -------------
            xrh_nm = [nm_pool.tile([128, BF], MMDT, name="nmt", tag="nm")
                      for _ in range(nt)]
            xrh_fm = [fm_pool.tile([128, N], MMDT, name="fmt", tag="fm") for _ in range(6)]
            for b in range(C):
                accr = gate_pool.tile([128, N], F32, name="gatet", tag="gate")
                nc.scalar.dma_start(accr[:], ACCR[b])
                r = gate_pool.tile([128, N], F32, name="gatet", tag="gate")
                nc.scalar.activation(r[:], accr[:], AF.Sigmoid, bias=brt[:])
                h = gate_pool.tile([128, N], F32, name="gatet", tag="gate")
                nc.scalar.dma_start(h[:], HFM[b])
                rh = fm_pool.tile([128, N], MMDT, name="fmt", tag="fm")
                nc.vector.tensor_mul(rh[:], r[:], h[:])
                # rh columns of xrh_nm (PE transpose 128-blocks)
                for g in range(nt // 4):
                    ps = psx_pool.tile([128, 512], MMDT, name="psxt", tag="psx")
                    for q in range(4):
                        it = 4 * g + q
                        nc.tensor.transpose(
                            ps[:, 128 * q:128 * (q + 1)],
                            rh[:, 128 * it:128 * (it + 1)], ident[:])
                    for q in range(4):
                        nc.vector.tensor_copy(
                            xrh_nm[4 * g + q][:, b * FI + 64:(b + 1) * FI],
                            ps[:, 128 * q:128 * (q + 1)])
                # fm rows of xrh: x piece then two rh 64-row pieces
                t, off = divmod(b * FI, 128)
                nc.scalar.dma_start(xrh_fm[t][off:off + 64, :], XFM[b])
                for s2 in range(2):
                    t, off = divmod(b * FI + 64 + 64 * s2, 128)
                    nc.scalar.dma_start(xrh_fm[t][off:off + 64, :],
                                        rh[64 * s2:64 * (s2 + 1), :])
            # x columns of xrh_nm straight from the xh param
            for jt in range(nt):
                for b in range(C):
                    nc.scalar.dma_start(xrh_nm[jt][:, b * FI:b * FI + 64],
                                        XH[jt][:, b * FI:b * FI + 64])
            # spill xrh_nm for the backward-chain reload
            for jt in range(nt):
                nc.sync.dma_start(XRH[jt], xrh_nm[jt][:])

            # ---------------- diffusion 2 (n gate) ----------------
            diffusion(lambda: load_nm(XRH), xrh_fm, [(WNI, ACCN)],
                      xnm_first=xrh_nm)

            # ---------------- final gate ----------------
            for b in range(C):
                accn = gate_pool.tile([128, N], F32, name="gatet", tag="gate")
                nc.scalar.dma_start(accn[:], ACCN[b])
                n_t = gate_pool.tile([128, N], F32, name="gatet", tag="gate")
                nc.scalar.activation(n_t[:], accn[:], AF.Tanh, bias=bnt[:])
                h = gate_pool.tile([128, N], F32, name="gatet", tag="gate")
                nc.scalar.dma_start(h[:], HFM[b])
                accz = gate_pool.tile([128, N], F32, name="gatet", tag="gate")
                nc.scalar.dma_start(accz[:], ACCZ[b])
                z = gate_pool.tile([128, N], F32, name="gatet", tag="gate")
                nc.scalar.activation(z[:], accz[:], AF.Sigmoid, bias=bzt[:])
                d = gate_pool.tile([128, N], F32, name="gatet", tag="gate")
                nc.vector.tensor_sub(d[:], n_t[:], h[:])
                zd2 = gate_pool.tile([128, N], F32, name="gatet", tag="gate")
                nc.vector.tensor_mul(zd2[:], z[:], d[:])
                o = gate_pool.tile([128, N], F32, name="gatet", tag="gate")
                nc.vector.tensor_add(o[:], zd2[:], h[:])
                nc.scalar.dma_start(OUT[b], o[:])

    nc.compile()
    return nc


def _pack_interleaved(W):
    """[128, 7*192] torch-Linear weight -> [7, 3, 64, 128] transposed 64-row
    contraction segments: out[k, s, f, o] = W[o, k*192 + 64*s + f]."""
    out = np.zeros((7, 3, 64, 128), np.float32)
    for k in range(7):
        for s in range(3):
            out[k, s] = W[:, k * FI + 64 * s:k * FI + 64 * (s + 1)].T
    return np.ascontiguousarray(out)


_NC_CACHE = {}


def _get_nc(nt):
    if nt not in _NC_CACHE:
        _NC_CACHE[nt] = build_nc(nt)
    return _NC_CACHE[nt]


def make_in_maps(x, h_prev, W_fwd, W_bwd, Wr, br, Wz, bz, Wn, bn):
    mdt = np.dtype(ml_dtypes.bfloat16) if MM_BF16 else np.float32
    x = np.asarray(x, np.float32)
    h_prev = np.asarray(h_prev, np.float32)
    B, N, Din = x.shape
    nt = N // 128
    WfT = np.ascontiguousarray(np.asarray(W_fwd, np.float32).T).reshape(nt, 128, N)
    WbT = np.ascontiguousarray(np.asarray(W_bwd, np.float32).T).reshape(nt, 128, N)
    wrI = _pack_interleaved(np.asarray(Wr, np.float32))
    wzI = _pack_interleaved(np.asarray(Wz, np.float32))
    wnI = _pack_interleaved(np.asarray(Wn, np.float32))
    ident = np.ascontiguousarray(np.eye(128, dtype=np.float32))
    WfT_d = WfT.astype(mdt)
    WbT_d = WbT.astype(mdt)
    wrI_d = wrI.astype(mdt)
    wzI_d = wzI.astype(mdt)
    wnI_d = wnI.astype(mdt)
    ident_d = ident.astype(mdt)
    brc = np.ascontiguousarray(np.asarray(br, np.float32).reshape(128, 1))
    bzc = np.ascontiguousarray(np.asarray(bz, np.float32).reshape(128, 1))
    bnc = np.ascontiguousarray(np.asarray(bn, np.float32).reshape(128, 1))
    ncores = B // C
    in_maps = []
    for cix in range(ncores):
        xs = x[C * cix:C * (cix + 1)]
        hs = h_prev[C * cix:C * (cix + 1)]
        xh = np.concatenate([xs, hs], axis=-1)            # [C, N, 192]
        flat = np.ascontiguousarray(xh.transpose(1, 0, 2).reshape(N, BF))
        xh_nm = np.ascontiguousarray(flat).reshape(nt, 128, BF)
        xh_fm = np.ascontiguousarray(flat.T).reshape(6, 128, N)
        x_fm = np.ascontiguousarray(xs.transpose(0, 2, 1))
        h_fm = np.ascontiguousarray(hs.transpose(0, 2, 1))
        in_maps.append(dict(
            xh_nm=xh_nm.astype(mdt), xh_fm=xh_fm.astype(mdt),
            wfT=WfT_d, wbT=WbT_d, wrI=wrI_d, wzI=wzI_d, wnI=wnI_d,
            x_fm=x_fm.astype(mdt), h_fm=h_fm,
            br_c=brc, bz_c=bzc, bn_c=bnc, ident=ident_d))
    return in_maps, nt, ncores


def kernel(x, h_prev, W_fwd, W_bwd, Wr, br, Wz, bz, Wn, bn, _trace=False):
    in_maps, nt, ncores = make_in_maps(
        x, h_prev, W_fwd, W_bwd, Wr, br, Wz, bz, Wn, bn)
    nc = _get_nc(nt)
    res = run_bass_kernel_spmd(nc, in_maps, list(range(ncores)), trace=_trace)
    outs = [np.ascontiguousarray(res.results[c]["out_fm"].transpose(0, 2, 1))
            for c in range(ncores)]
    full = np.concatenate(outs, axis=0).astype(np.float32)
    if _trace:
        return full, res
    return full

